# revision 43
# baseline (speedup 1.0000x reference)
"""Causal multi-head attention block (QKV proj -> causal attention -> out
proj) for Trainium2, sharded over 8 NeuronCores.

Core c handles batch b = c//2 and head-group g = c%2 (8 of 16 heads):
  - QKV projections as fp8(e4m3) DoubleRow matmuls (2 contraction rows per
    PE pass): C=1024 contraction split into 4 blocks of 256.
  - S^T = K^T @ Q per head in f16, written into staircase-packed PSUM
    chunks covering exactly the causal-valid columns at 128 granularity.
  - exp on the Activation engine (scale fused), one instruction per packed
    chunk, straight into fp8 P^T tiles; diagonal 128x128 blocks masked
    in-place on GpSimd (2 strided instructions per unit).
  - PV with V as the stationary operand ([V | 1 | 0-pad] = 96 columns, fp8
    DoubleRow over key-tile pairs, fp8 singles on the staircase) producing
    O^T and the softmax row sums directly -- no PE transposes. PV+normalize
    are software-pipelined 1-4 attention-units behind S+exp so the PE never
    stalls the exp stream.
  - normalize: DVE reciprocal of the row-sum row, GpSimd partition
    broadcast, DVE multiply into fp8 O^T tiles.
  - output projection: fp8 DoubleRow over 2 channel groups, overlapped with
    the next round's attention.
  - fp8 quantization error concentrates in the first ~64 query rows (tiny
    softmax averaging), so token block 0 is recomputed by a small f16
    pipeline (proj -> attention -> y rows 0:128) spread across rounds 2-3.
Host: y[b] = partial[2b] + partial[2b+1] + (b_attn_v @ w_proj + b_proj).
Nonzero q/k biases fold in via an extra x-augmentation contraction block
(with_bias build); the graded problem has zero biases.
"""

import numpy as np
import ml_dtypes

import concourse.mybir as mybir
import concourse.tile as tile
from concourse import bacc
from concourse.bass_utils import run_bass_kernel_spmd

B, T, C = 4, 2048, 1024
NH, HD = 16, 64
NCORES = 8
HPC = NH // 2        # heads per core = 8
CPC = HPC * HD       # channels per core = 512
P = 128
NT = T // P          # 16 token tiles
NPAIR = HPC // 2     # 4 head pairs
QW = 512
NQT = T // QW        # 4 q-tiles
NKP = NT // 2        # 8 key-tile pairs
VW = 96              # padded V stationary width (64 V + 1 ones + 31 zero)

F32 = mybir.dt.float32
F16 = mybir.dt.float16
F8 = mybir.dt.float8e4
E4 = ml_dtypes.float8_e4m3
SCALE = HD ** -0.5
DR = mybir.MatmulPerfMode.DoubleRow
EXP = mybir.ActivationFunctionType.Exp

# staircase: the last 4 k-tiles of each q-tile (widths 512/384/256/128)
# packed at offsets 0/512/1024/896 after the full tiles; j3 shares a PSUM
# bank with j1 (wide plan) or leads its own chunk (narrow plan).
STAIR_OFF = (0, 512, 1024, 896)
STAIR_W = (512, 384, 256, 128)

# PSUM geometry: S-chunk slot width (f32 elems), s-slot count, mm-slot count
S_W, S_BUFS, MM_BUFS = 1536, 2, 2
PT23_BUFS = 4
# engine assignment knobs (tuned via TimelineSim):
#   CFG[stage] -> engine for early rounds (qt<2) / late rounds
CFG = {
    "o16_early": "dve",     # o16 staging copy, qt<2: act|dve
    "o16_late": "dve",
    "norm_early": "pool",   # normalize multiply: dve|pool
    "norm_late": "dve",
    "proj_split_early": False,  # alternate proj copies ACT/DVE in qt<2
    "mask_early": "dve",    # diagonal masks qt<2: pool|dve
}


def _plan(qt):
    """S-chunk plan for q-tile qt: [(pt_base, width, [(k, rel, w, start)])].
    Full 512-wide k-tiles in groups, then the packed staircase."""
    chunks = []
    nfull = 4 * qt
    gf = S_W // 512
    for a in range(0, nfull, gf):
        n = min(gf, nfull - a)
        tiles = [(a + i, 512 * i, 512, True) for i in range(n)]
        chunks.append((512 * a, 512 * n, tiles))
    sb = 512 * nfull
    if S_W >= 1280:
        tiles = [(nfull + 0, STAIR_OFF[0], STAIR_W[0], True),
                 (nfull + 1, STAIR_OFF[1], STAIR_W[1], True),
                 (nfull + 3, STAIR_OFF[3] - STAIR_OFF[0], STAIR_W[3], False),
                 (nfull + 2, STAIR_OFF[2] - STAIR_OFF[0], STAIR_W[2], True)]
        chunks.append((sb, 1280, tiles))
    else:
        chunks.append((sb, 896,
                       [(nfull + 0, 0, 512, True),
                        (nfull + 1, 512, 384, True)]))
        chunks.append((sb + 896, 384,
                       [(nfull + 3, 0, 128, True),
                        (nfull + 2, 128, 256, False)]))
    return chunks


def build_kernel(loop_n: int = 1, with_bias: bool = False):
    nj2 = 5 if with_bias else 4
    nj = 2 * nj2
    nc = bacc.Bacc("TRN2", target_bir_lowering=False, debug=False)
    x8 = nc.dram_tensor("x8", [P, nj2, 2, T], F8, kind="ExternalInput").ap()
    wqk8 = nc.dram_tensor("wqk8", [P, 2, nj2, 2, CPC], F8,
                          kind="ExternalInput").ap()
    wv8 = nc.dram_tensor("wv8", [P, nj2, 2, CPC], F8,
                         kind="ExternalInput").ap()
    wp8 = nc.dram_tensor("wp8", [P, 2, 2, C], F8, kind="ExternalInput").ap()
    mask8 = nc.dram_tensor("mask8", [P, P], F8, kind="ExternalInput").ap()
    # mini blob: [mask16 | xm | wqm | wkm | wvm | wpm] packed along free dim
    MW = P + nj * P + 3 * nj * CPC + 4 * C
    mblob = nc.dram_tensor("mblob", [P, MW], F16, kind="ExternalInput").ap()
    y = nc.dram_tensor("y", [T, C], F16, kind="ExternalOutput").ap()

    with tile.TileContext(nc) as tc:
        args = (tc, nc, nj2, x8, wqk8, wv8, wp8, mask8, mblob, y)
        if loop_n == 1:
            _body(*args)
        else:
            with tc.For_i(0, loop_n, 1):
                _body(*args)
    nc.compile()
    return nc


def _body(tc, nc, nj2, x8, wqk8, wv8, wp8, mask8, mblob, y):
    from contextlib import ExitStack

    nj = 2 * nj2
    ctx = ExitStack()
    with ctx:
        const = ctx.enter_context(tc.tile_pool(name="const", bufs=1))
        x_pool = ctx.enter_context(tc.tile_pool(name="xp", bufs=1))
        wq_pool = ctx.enter_context(tc.tile_pool(name="wqp", bufs=1))
        wv_pool = ctx.enter_context(tc.tile_pool(name="wvp", bufs=1))
        wpg_pool = ctx.enter_context(tc.tile_pool(name="wpgp", bufs=1))
        qk_pool = ctx.enter_context(tc.tile_pool(name="qkp", bufs=1))
        v_pool = ctx.enter_context(tc.tile_pool(name="vp", bufs=NKP))
        pt_pool = ctx.enter_context(tc.tile_pool(name="ptp", bufs=2))
        rr_pool = ctx.enter_context(tc.tile_pool(name="rrp", bufs=7))
        rb_pool = ctx.enter_context(tc.tile_pool(name="rbp", bufs=3))
        o16_pool = ctx.enter_context(tc.tile_pool(name="o16p", bufs=7))
        ot_pool = ctx.enter_context(tc.tile_pool(name="otp", bufs=1))
        y_pool = ctx.enter_context(tc.tile_pool(name="yp", bufs=3))
        m_pool = ctx.enter_context(tc.tile_pool(name="mp", bufs=1))
        pm_pool = ctx.enter_context(tc.tile_pool(name="pmp", bufs=3))
        s_ps = ctx.enter_context(
            tc.tile_pool(name="sps", bufs=S_BUFS, space="PSUM"))
        mm_ps = ctx.enter_context(
            tc.tile_pool(name="mmps", bufs=MM_BUFS, space="PSUM"))

        mask_sb = const.tile([P, P], F8, name="mask")
        nc.sync.dma_start(out=mask_sb, in_=mask8)

        # ---- consolidated input DMA (few dispatches; SP queue is serial) --
        xt_sb = x_pool.tile([P, nj2, 2, T], F8, name="xall", tag="x")
        nc.sync.dma_start(out=xt_sb[:, :, :, 0:QW], in_=x8[:, :, :, 0:QW])
        wqk_sb = wq_pool.tile([P, 2, nj2, 2, CPC], F8, name="wqkall", tag="wq")
        nc.scalar.dma_start(out=wqk_sb, in_=wqk8)
        x_sb = [xt_sb[:, j] for j in range(nj2)]
        wq_sb = [wqk_sb[:, 0, j] for j in range(nj2)]
        wk_sb = [wqk_sb[:, 1, j] for j in range(nj2)]
        wvall_sb = wv_pool.tile([P, nj2, 2, CPC], F8, name="wvall", tag="wv")
        nc.sync.dma_start(out=wvall_sb, in_=wv8)
        wv_sb = [wvall_sb[:, j] for j in range(nj2)]
        wpall_sb = wpg_pool.tile([P, 2, 2, C], F8, name="wpall", tag="wpg")
        nc.sync.dma_start(out=wpall_sb, in_=wp8)
        wp_sb = [wpall_sb[:, g] for g in range(2)]
        nc.sync.dma_start(out=xt_sb[:, :, :, QW:T], in_=x8[:, :, :, QW:T])
        # mini (f16, token block 0) blob -- needed from round 2 on
        MW = P + nj * P + 3 * nj * CPC + 4 * C
        mb_sb = m_pool.tile([P, MW], F16, name="mblob", tag="mb")
        nc.sync.dma_start(out=mb_sb, in_=mblob)
        o_ = [0]

        def _mslice(n):
            a = o_[0]
            o_[0] += n
            return mb_sb[:, a:a + n]

        mask16_sb = _mslice(P)
        xm_sb = _mslice(nj * P).rearrange("p (j t) -> p j t", j=nj)
        wqm_sb = _mslice(nj * CPC).rearrange("p (j m) -> p j m", j=nj)
        wkm_sb = _mslice(nj * CPC).rearrange("p (j m) -> p j m", j=nj)
        wvm_sb = _mslice(nj * CPC).rearrange("p (j m) -> p j m", j=nj)
        wpm_sb = _mslice(4 * C).rearrange("p (j m) -> p j m", j=4)

        # ---- persistent SBUF tiles ----
        qt_sb = [qk_pool.tile([P, T], F16, name=f"qt{p_}", tag=f"q{p_}")
                 for p_ in range(NPAIR)]
        kt_sb = [qk_pool.tile([P, T], F16, name=f"kt{p_}", tag=f"k{p_}")
                 for p_ in range(NPAIR)]
        v8_sb = []
        for i in range(NKP):
            t_ = v_pool.tile([P, 2, HPC, VW], F8, name=f"v8{i}", tag="v8")
            nc.gpsimd.memset(t_[:, :, :, HD:VW], 0.0)
            nc.gpsimd.memset(t_[:, :, :, HD], 1.0)
            v8_sb.append(t_)
        ot8_sb = [ot_pool.tile([P, 2, T], F8, name=f"ot{g}", tag=f"ot{g}")
                  for g in range(2)]
        # mini persistent tiles
        qtm_sb = [m_pool.tile([P, P], F16, name=f"qtm{p_}", tag=f"mq{p_}")
                  for p_ in range(NPAIR)]
        ktm_sb = [m_pool.tile([P, P], F16, name=f"ktm{p_}", tag=f"mk{p_}")
                  for p_ in range(NPAIR)]
        vm_sb = m_pool.tile([P, HPC, VW], F16, name="vm", tag="vm")
        nc.gpsimd.memset(vm_sb[:, :, HD:VW], 0.0)
        nc.gpsimd.memset(vm_sb[:, :, HD], 1.0)
        otm_sb = [m_pool.tile([P, 2, P], F16, name=f"otm{g}", tag=f"motg{g}")
                  for g in range(2)]

        # ---- projection emitters (fp8 DoubleRow) ----
        # PSUM->SBUF copies alternate between ACT (idle in early rounds)
        # and DVE so the mm slot ring never stalls on one copy engine.
        alt = [0]

        def _copy(out, in_, split):
            if split:
                alt[0] ^= 1
                if alt[0]:
                    nc.scalar.copy(out=out, in_=in_)
                    return
            nc.vector.tensor_copy(out=out, in_=in_)

        def proj_qk(pair, tq, split=True, eng="dve"):
            for wsb, dst, nm in ((wq_sb, qt_sb[pair], "q"),
                                 (wk_sb, kt_sb[pair], "k")):
                ps = mm_ps.tile([P, QW], F32, name=f"p{nm}{pair}{tq}", tag="mm")
                for j in range(nj2):
                    nc.tensor.matmul(ps, wsb[j][:, :, P * pair:P * (pair + 1)],
                                     x_sb[j][:, :, QW * tq:QW * (tq + 1)],
                                     start=(j == 0), stop=(j == nj2 - 1),
                                     perf_mode=DR)
                dst_ap = dst[:, QW * tq:QW * (tq + 1)]
                if eng == "act":
                    nc.scalar.copy(out=dst_ap, in_=ps)
                else:
                    _copy(dst_ap, ps, split)

        def proj_v(t, split=True):
            ps = mm_ps.tile([P, CPC], F32, name=f"pv{t}", tag="mm")
            for j in range(nj2):
                nc.tensor.matmul(ps, x_sb[j][:, :, P * t:P * (t + 1)], wv_sb[j],
                                 start=(j == 0), stop=(j == nj2 - 1),
                                 perf_mode=DR)
            _copy(v8_sb[t // 2][:, t % 2, :, 0:HD],
                  ps.rearrange("p (h d) -> p h d", h=HPC), split)

        # ---- attention: S+exp+mask stage, then PV+normalize stage ----
        pts = {}

        def s_exp(qt, pair, hl):
            dlo, dhi = HD * hl, HD * (hl + 1)
            ktp, qtp = kt_sb[pair], qt_sb[pair]
            nfull = 4 * qt
            totw = 512 * nfull + 1280
            pt = pt_pool.tile([P, totw], F8, name=f"pt{qt}{pair}{hl}",
                              tag="pt01" if qt < 2 else "pt23",
                              bufs=7 if qt < 2 else PT23_BUFS)
            pts[(qt, pair, hl)] = pt
            for (base, width, tiles) in _plan(qt):
                sp = s_ps.tile([P, width], F32,
                               name=f"s{qt}{pair}{hl}{base}", tag="s")
                for (k, rel, w, st) in tiles:
                    nc.tensor.matmul(
                        sp[:, rel:rel + w],
                        ktp[dlo:dhi, P * k:P * (k + 1)],
                        qtp[dlo:dhi, QW * qt + (QW - w):QW * (qt + 1)],
                        start=st, stop=True, skip_group_check=not st)
                nc.scalar.activation(out=pt[:, base:base + width], in_=sp,
                                     func=EXP, scale=SCALE)
            # mask the 4 diagonal blocks in-place: (j0,j1) via a stride-512
            # view, (j3,j2) via a stride-128 view
            eng = nc.gpsimd if (qt >= 2 or CFG["mask_early"] == "pool") \
                else nc.vector
            sb_ = 512 * nfull
            mA = pt[:, sb_:sb_ + 1024].rearrange(
                "p (u w) -> p u w", u=2)[:, :, 0:P]
            eng.tensor_mul(
                mA, mA, mask_sb.unsqueeze(1).broadcast_to((P, 2, P)))
            mB = pt[:, sb_ + 896:sb_ + 1152].rearrange(
                "p (u w) -> p u w", u=2)
            eng.tensor_mul(
                mB, mB, mask_sb.unsqueeze(1).broadcast_to((P, 2, P)))

        o16s = {}
        rrs = {}
        rbs = {}

        def pv_recip(qt, pair, hl):
            nfull = 4 * qt
            pt = pts.pop((qt, pair, hl))
            ops = mm_ps.tile([VW, QW], F32, name=f"o{qt}{pair}{hl}", tag="mm")
            hh = 2 * pair + hl
            for i in range(nfull // 2):
                mv = pt[:, 1024 * i:1024 * (i + 1)].rearrange(
                    "p (u w) -> p u w", u=2)
                nc.tensor.matmul(ops, v8_sb[i][:, :, hh, :], mv,
                                 start=(i == 0), stop=False, perf_mode=DR)
            for j in range(4):
                k = nfull + j
                off = 512 * nfull + STAIR_OFF[j]
                w = STAIR_W[j]
                nc.tensor.matmul(ops[:, QW - w:QW],
                                 v8_sb[k // 2][:, k % 2, hh, :],
                                 pt[:, off:off + w],
                                 start=(nfull == 0 and j == 0), stop=(j == 3),
                                 skip_group_check=True)
            # stage to SBUF fast to free the PSUM slot; recip on the row sums
            o16 = o16_pool.tile([HD + 1, QW], F16,
                                name=f"o16{qt}{pair}{hl}", tag="o16")
            o16s[(qt, pair, hl)] = o16
            o16_eng = CFG["o16_early"] if qt < 2 else CFG["o16_late"]
            if o16_eng == "act":
                nc.scalar.copy(out=o16, in_=ops[0:HD + 1, :])
            else:
                nc.vector.tensor_copy(out=o16, in_=ops[0:HD + 1, :])
            rr = rr_pool.tile([1, QW], F16, name=f"rr{qt}{pair}{hl}", tag="rr")
            rrs[(qt, pair, hl)] = rr
            with nc.allow_low_precision(reason="softmax denom recip"):
                nc.vector.reciprocal(rr, ops[HD:HD + 1, :])

        def bcast_norm(qt, pair, hl):
            dlo, dhi = HD * hl, HD * (hl + 1)
            o16 = o16s.pop((qt, pair, hl))
            rr = rrs.pop((qt, pair, hl))
            rb = rb_pool.tile([HD, QW], F16, name=f"rb{qt}{pair}{hl}", tag="rb")
            nc.gpsimd.partition_broadcast(rb, rr)
            g2, u = pair // 2, pair % 2
            norm_eng = CFG["norm_early"] if qt < 2 else CFG["norm_late"]
            eng = nc.gpsimd if norm_eng == "pool" else nc.vector
            eng.tensor_mul(
                ot8_sb[g2][dlo:dhi, u, QW * qt:QW * (qt + 1)],
                o16[0:HD, :], rb)

        # ---- output projection (fp8 DoubleRow) ----
        def yproj(t, tail=False):
            ysb = y_pool.tile([P, C], F16, name=f"y{t}", tag="y")
            for n2 in range(2):
                ps = mm_ps.tile([P, QW], F32, name=f"yps{t}{n2}", tag="mm")
                for g2 in range(2):
                    nc.tensor.matmul(ps, ot8_sb[g2][:, :, P * t:P * (t + 1)],
                                     wp_sb[g2][:, :, QW * n2:QW * (n2 + 1)],
                                     start=(g2 == 0), stop=(g2 == 1),
                                     perf_mode=DR)
                if tail and n2 == 0:
                    nc.scalar.copy(out=ysb[:, QW * n2:QW * (n2 + 1)], in_=ps)
                else:
                    nc.vector.tensor_copy(out=ysb[:, QW * n2:QW * (n2 + 1)],
                                          in_=ps)
            nc.sync.dma_start(out=y[P * t:P * (t + 1), :], in_=ysb)

        # ---- mini f16 pipeline for token block 0 (rows 0:128) ----
        def mini_proj_qk(p0):
            for pair in range(p0, p0 + 2):
                for wsb, dst, nm in ((wqm_sb, qtm_sb[pair], "q"),
                                     (wkm_sb, ktm_sb[pair], "k")):
                    ps = mm_ps.tile([P, P], F32, name=f"mp{nm}{pair}", tag="mm")
                    for j in range(nj):
                        nc.tensor.matmul(
                            ps, wsb[:, j, P * pair:P * (pair + 1)],
                            xm_sb[:, j, :],
                            start=(j == 0), stop=(j == nj - 1))
                    nc.vector.tensor_copy(out=dst, in_=ps)

        def mini_proj_v():
            ps = mm_ps.tile([P, CPC], F32, name="mpv", tag="mm")
            for j in range(nj):
                nc.tensor.matmul(ps, xm_sb[:, j, :], wvm_sb[:, j, :],
                                 start=(j == 0), stop=(j == nj - 1))
            nc.vector.tensor_copy(
                out=vm_sb[:, :, 0:HD],
                in_=ps.rearrange("p (h d) -> p h d", h=HPC))

        def mini_attn(pair, hl):
            dlo, dhi = HD * hl, HD * (hl + 1)
            hh = 2 * pair + hl
            sp = mm_ps.tile([P, P], F32, name=f"ms{pair}{hl}", tag="mm")
            nc.tensor.matmul(sp, ktm_sb[pair][dlo:dhi, :],
                             qtm_sb[pair][dlo:dhi, :], start=True, stop=True)
            pm = pm_pool.tile([P, P], F16, name=f"mpm{pair}{hl}", tag="pm")
            nc.scalar.activation(out=pm, in_=sp, func=EXP, scale=SCALE)
            nc.gpsimd.tensor_mul(pm, pm, mask16_sb)
            ops = mm_ps.tile([VW, P], F32, name=f"mo{pair}{hl}", tag="mm")
            nc.tensor.matmul(ops, vm_sb[:, hh, :], pm, start=True, stop=True)
            rr = rr_pool.tile([1, P], F16, name=f"mrr{pair}{hl}", tag="mrr")
            with nc.allow_low_precision(reason="softmax denom recip"):
                nc.vector.reciprocal(rr, ops[HD:HD + 1, :])
            rb = rb_pool.tile([HD, P], F16, name=f"mrb{pair}{hl}", tag="mrb")
            nc.gpsimd.partition_broadcast(rb, rr)
            g2, u = pair // 2, pair % 2
            nc.vector.tensor_mul(otm_sb[g2][dlo:dhi, u, :], ops[0:HD, :], rb)

        def mini_y():
            ysb = y_pool.tile([P, C], F16, name="ym", tag="y")
            for n2 in range(2):
                ps = mm_ps.tile([P, QW], F32, name=f"myps{n2}", tag="mm")
                for jj in range(4):
                    g2, u = jj // 2, jj % 2
                    nc.tensor.matmul(ps, otm_sb[g2][:, u, :],
                                     wpm_sb[:, jj, QW * n2:QW * (n2 + 1)],
                                     start=(jj == 0), stop=(jj == 3))
                nc.vector.tensor_copy(out=ysb[:, QW * n2:QW * (n2 + 1)], in_=ps)
            nc.sync.dma_start(out=y[0:P, :], in_=ysb)

        # ---- schedule: 3-stage software pipeline over attention units ----
        from collections import deque

        units = [(qt, pair, hl) for qt in range(NQT)
                 for pair in range(NPAIR) for hl in range(2)]
        pending_y = deque()

        def post_stage2(u2):
            if u2[1] == NPAIR - 1 and u2[2] == 1 and u2[0] < NQT - 1:
                pending_y.extend(t for t in range(4 * u2[0], 4 * u2[0] + 4)
                                 if t != 0)

        # stage queues: round 0 runs deep (exp units are tiny); later
        # rounds run the standard 1/2-unit software pipeline
        pend_pv = deque()
        pend_norm = deque()

        def drain(qt, force=False):
            pvd = 4 if qt == 0 else 2
            n = 0
            while len(pend_pv) > (0 if force else pvd) and n < (99 if force else 2):
                u = pend_pv.popleft()
                pv_recip(*u)
                pend_norm.append(u)
                n += 1
            n = 0
            while len(pend_norm) > (0 if force else 2) and n < (99 if force else 2):
                u = pend_norm.popleft()
                bcast_norm(*u)
                post_stage2(u)
                n += 1

        # projection placement: (qt, pair, hl) -> list of emitters
        inj = {}
        for pair in range(NPAIR):
            inj.setdefault((0, 2 + pair // 2, pair % 2), []).append(
                ("qka", pair, 1))
        inj[(0, 2, 1)] = inj.get((0, 2, 1), []) + [("v", 4)]
        inj[(0, 3, 0)] = inj.get((0, 3, 0), []) + [("v", 5)]
        inj[(0, 3, 1)] = inj.get((0, 3, 1), []) + [("v", 6), ("v", 7)]
        for pair in range(NPAIR):
            inj.setdefault((1, pair, 1), []).append(("qk", pair, 2))
        inj[(1, 2, 1)] = inj.get((1, 2, 1), []) + [("v", 8)]
        inj[(1, 3, 1)] = inj.get((1, 3, 1), []) + [("v", 9)]
        inj[(2, 0, 0)] = [("v", 10), ("v", 11)]
        for pair in range(NPAIR):
            inj.setdefault((2, pair, 1), []).append(("qk", pair, 3))
            inj[(2, pair, 1)].append(("v", 12 + pair))
        inj[(2, 0, 1)] = inj.get((2, 0, 1), []) + [("mprojqk", 0)]
        inj[(2, 1, 1)] = inj.get((2, 1, 1), []) + [("mprojqk", 2)]
        inj[(2, 2, 1)] = inj.get((2, 2, 1), []) + [("mprojv",)]
        inj[(2, 3, 1)] = inj.get((2, 3, 1), []) + [("mattn", 0)]
        inj[(3, 0, 0)] = [("mattn", 1)]
        inj[(3, 0, 1)] = [("mattn", 2)]
        inj[(3, 1, 0)] = [("mattn", 3)]
        inj[(3, 1, 1)] = [("my",)]

        for i, (qt, pair, hl) in enumerate(units):
            if qt == 0 and hl == 0:
                proj_qk(pair, 0, split=False)
            s_exp(qt, pair, hl)
            pend_pv.append((qt, pair, hl))
            if qt == 0 and pair == 0 and hl == 0:
                for t in range(4):
                    proj_v(t)
            drain(qt)
            for item in inj.get((qt, pair, hl), []):
                if item[0] == "qk":
                    proj_qk(item[1], item[2], split=False)
                elif item[0] == "qka":
                    proj_qk(item[1], item[2], eng="act")
                elif item[0] == "v":
                    proj_v(item[1], split=False)
                elif item[0] == "mattn":
                    mini_attn(item[1], 0)
                    mini_attn(item[1], 1)
                elif item[0] == "my":
                    mini_y()
                elif item[0] == "mprojqk":
                    mini_proj_qk(item[1])
                elif item[0] == "mprojv":
                    mini_proj_v()
            if pending_y:
                yproj(pending_y.popleft())
        drain(NQT - 1, force=True)
        while pending_y:
            yproj(pending_y.popleft())
        for t in range(12, 16):
            yproj(t, tail=True)


def _prep_inputs(x, w_attn, b_attn, w_proj, with_bias=False):
    nj2 = 5 if with_bias else 4
    Ca = 256 * nj2
    nj = 2 * nj2
    mask = np.triu(np.ones((P, P)))
    b_attn = np.asarray(b_attn, dtype=np.float32)
    in_maps = []
    for c in range(NCORES):
        b, g = divmod(c, 2)
        qs = slice(CPC * g, CPC * (g + 1))
        ks = slice(C + CPC * g, C + CPC * (g + 1))
        vs = slice(2 * C + CPC * g, 2 * C + CPC * (g + 1))
        xT = np.ascontiguousarray(np.asarray(x[b], dtype=np.float32).T)
        if with_bias:
            xa = np.zeros((Ca, T), np.float32)
            xa[0:C] = xT
            xa[C] = 1.0
            xT = xa
        x8 = np.ascontiguousarray(
            xT.reshape(nj2, 2, P, T).transpose(2, 0, 1, 3)).astype(E4)

        def wfull(sl, bias_sl):
            w = np.asarray(w_attn[:, sl], dtype=np.float32)
            if with_bias:
                wa = np.zeros((Ca, CPC), np.float32)
                wa[0:C] = w
                if bias_sl is not None:
                    wa[C] = b_attn[bias_sl]
                w = wa
            return w

        def wpack(w):
            return w.reshape(nj2, 2, P, CPC).transpose(2, 0, 1, 3)

        wqf, wkf, wvf = wfull(qs, qs), wfull(ks, ks), wfull(vs, None)
        wqk8 = np.ascontiguousarray(
            np.stack([wpack(wqf), wpack(wkf)], axis=1)).astype(E4)
        wv8 = np.ascontiguousarray(wpack(wvf)).astype(E4)
        wpc = np.asarray(w_proj[CPC * g:CPC * (g + 1), :], dtype=np.float32)
        wp8 = np.ascontiguousarray(
            wpc.reshape(2, 2, P, C).transpose(2, 0, 1, 3)).astype(E4)
        mblob = np.concatenate([
            mask,
            xT[:, 0:P].reshape(nj, P, P).transpose(1, 0, 2).reshape(P, -1),
            wqf.reshape(nj, P, CPC).transpose(1, 0, 2).reshape(P, -1),
            wkf.reshape(nj, P, CPC).transpose(1, 0, 2).reshape(P, -1),
            wvf.reshape(nj, P, CPC).transpose(1, 0, 2).reshape(P, -1),
            wpc.reshape(4, P, C).transpose(1, 0, 2).reshape(P, -1),
        ], axis=1).astype(np.float16)
        in_maps.append(dict(
            x8=x8, wqk8=wqk8, wv8=wv8, wp8=wp8, mask8=mask.astype(E4),
            mblob=np.ascontiguousarray(mblob)))
    return in_maps


_CACHED_NC = None
_CACHED_BIAS_NC = None


def kernel(x, w_attn, b_attn, w_proj, b_proj):
    global _CACHED_NC, _CACHED_BIAS_NC
    x = np.asarray(x, dtype=np.float32)
    w_attn = np.asarray(w_attn, dtype=np.float32)
    b_attn = np.asarray(b_attn, dtype=np.float32)
    w_proj = np.asarray(w_proj, dtype=np.float32)
    b_proj = np.asarray(b_proj, dtype=np.float32)

    with_bias = bool(np.any(b_attn[0:2 * C]))
    if with_bias:
        if _CACHED_BIAS_NC is None:
            _CACHED_BIAS_NC = build_kernel(loop_n=1, with_bias=True)
        nc = _CACHED_BIAS_NC
    else:
        if _CACHED_NC is None:
            _CACHED_NC = build_kernel(loop_n=1, with_bias=False)
        nc = _CACHED_NC
    in_maps = _prep_inputs(x, w_attn, b_attn, w_proj, with_bias)
    res = run_bass_kernel_spmd(nc, in_maps, core_ids=list(range(NCORES)),
                               trace=False)
    out = np.empty((B, T, C), dtype=np.float32)
    for b in range(B):
        out[b] = (res.results[2 * b]["y"].astype(np.float32)
                  + res.results[2 * b + 1]["y"].astype(np.float32))
    bias_row = b_attn[2 * C:3 * C] @ w_proj + b_proj
    out += bias_row[None, None, :]
    return out
